# revision 18
# baseline (speedup 1.0000x reference)
"""Trainium2 Bass kernel for nn_ChannelLatencySeq2Value (B=8, C=256, T=4096).

Structure of the computation (derived analytically from the reference):
  * The 3 depthwise conv paths (k=3/5/9, out_per_kernel=6) followed by the
    grouped 1x1 reduce collapse into a single sparse conv:
        drive[b,c,t] = sum_{j<3} sum_{k<9} g[c,j,k] * x[b,(3c+j)%256, t+k-4]
    (the conv biases are zero).  g is composed on the host (tiny).
  * The LIF scan V = a*V + (1-a)*drive is a first-order linear recurrence ->
    tensor_tensor_scan on VectorE (fp32 state).
  * No neuron fires for the reference input distribution (max V ~ 0.76), so
    first-spike latency == T for every row.  The device certifies this with a
    soft-max detector A[c] = sum_t exp(LAM*(V[c,t]-1)); rows with
    A >= exp(LAM*(THR_DET-1)) (i.e. V possibly above THR_DET) are recomputed
    exactly on the host.  For the fixed input distribution this never
    triggers; it guarantees exactness if it ever does.
  * the tiny (B,C) MLP head runs on the host in fp32 (<< 0.01% of FLOPs).

Device work per core (data-parallel over batch, 1 batch element per core):
  * encoder as fp8(e4m3) DoubleRow matmuls on TensorE: DoubleRow virtualizes
    the PE array to 256 contraction rows at 0.5 cycles/row, so ALL 256 input
    channels contract in one matmul -- no channel-window splitting and no
    straddler fixups.  9 shift-matmuls (one per tap) accumulate one PSUM
    bank per (128-channel tile, 512-step time chunk).
  * fp8 weights are scaled per output channel by a power of two (keeps them
    out of the fp8 subnormal range); V is then scaled by 2^e[c], undone for
    free inside the ScalarE exp (per-partition scale operand).
  * VectorE runs the LIF scan directly from PSUM; ScalarE computes the
    exp-sum detector via its fused accumulator.  Both hide under the PE.
"""

import numpy as np
import ml_dtypes

import concourse.bass as bass
import concourse.bacc as bacc
import concourse.mybir as mybir
from concourse.tile import TileContext
from concourse.bass_utils import run_bass_kernel_spmd


def _ensure_axon_hooks():
    # bass_utils' BASS_TRACE path imports antenv.axon_hooks, which does not
    # exist in this image; provide a no-op stub so a stray BASS_TRACE env
    # var cannot crash the kernel (tracing is then skipped gracefully).
    try:
        import antenv.axon_hooks  # noqa: F401
    except ImportError:
        import sys
        import types
        m = types.ModuleType("antenv.axon_hooks")
        m.get_axon_ntff_profile_hook = lambda: None
        m.set_axon_ntff_profile_hook = lambda h: None
        sys.modules["antenv.axon_hooks"] = m


_ensure_axon_hooks()

# ---------------------------------------------------------------- constants
B, C, T = 8, 256, 4096
OP = 6
ALPHA = float(np.exp(-1.0 / 5.0))
OMA = 1.0 - ALPHA
THRESHOLD = 1.0
TC = 512                      # time chunk (= one PSUM bank of fp32)
NT = T // TC
PAD = 4                       # conv halo (kernel width 9)
XCOLS = 4112                  # 4 + 4096 + 12 (16-aligned half pitch)
NCORES = 8
NWARM = 6                     # PE p-state warmup matmuls during DMA-in
GRP = 4                       # chunks per weight-load group (LDW amortization)
POOL_SCANS = 0                # GpSimd cannot run scans on TRN2 (ISA check)
SWI = True                    # DoubleRowSwInterleave: software-interleaved
                              # weights -> contiguous (fast) LDWEIGHTS

LAM = 96.0                    # exp-sum detector sharpness
THR_DET = 0.90                # flag rows whose V may exceed this
ACC_THRESH = float(np.exp(LAM * (THR_DET - 1.0)))

F8 = ml_dtypes.float8_e4m3fn


def _compose_g(w3, b3, w5, b5, w9, b9, w_red, b_red):
    """Collapse the 4-conv encoder into g[c,3,9] (fp64 accum) + beta[c]."""
    g = np.zeros((C, 3, 9), np.float64)
    beta = np.zeros((C,), np.float64)
    paths = [(np.asarray(w3, np.float64), np.asarray(b3, np.float64), 3),
             (np.asarray(w5, np.float64), np.asarray(b5, np.float64), 5),
             (np.asarray(w9, np.float64), np.asarray(b9, np.float64), 9)]
    wr = np.asarray(w_red, np.float64)
    for c in range(C):
        beta[c] += float(b_red[c])
        for i in range(18):
            m = c * 18 + i
            wp, bp, K = paths[m // (C * OP)]
            q = m % (C * OP)
            s = q // OP
            j = (s - 3 * c) % 256
            assert j in (0, 1, 2)
            pad = (K - 1) // 2
            w = wr[c, i, 0]
            beta[c] += w * bp[q]
            g[c, j, 4 - pad:4 + pad + 1] += w * wp[q, 0, :]
    return g, beta


def _scale_exponents(gs):
    """Per-channel power-of-two e[c] so max|gs[c]*2^e| lands in (0.22, 0.44]:
    keeps every dominant fp8 weight in the normal range."""
    maxab = np.abs(gs).reshape(C, -1).max(axis=1)
    maxab = np.maximum(maxab, 1e-12)
    return np.floor(np.log2(0.4375 / maxab)).astype(np.int32)


def _build_weights(ws8):
    """DoubleRow lhsT stacks.  Logical weight L[p, ih, m] = ws8[c, j, k] for
    output channel c=ci*128+m whose input channel s=(3c+j)%256 sits at
    partition p=s%128 of half ih=s//128.
      SWI=False: wt[p, ci*9+k, ih, m] = L  (hardware-interleaved DoubleRow)
      SWI=True:  wt[p, ci*9+k, 2*(127-m)+ih] = L  (pre-interleaved; the PE
                 reads the 256 weight columns contiguously -> fast LDW)"""
    wt = np.zeros((128, 18, 2, 128), F8)
    cs = np.arange(C)
    for j in range(3):
        s = (3 * cs + j) % 256
        p, ih = s % 128, s // 128
        for k in range(9):
            ci = cs // 128
            m = cs % 128
            if SWI:
                col = 2 * (127 - m) + ih
                wt[p, ci * 9 + k, col // 128, col % 128] = ws8[cs, j, k]
            else:
                wt[p, ci * 9 + k, ih, m] = ws8[cs, j, k]
    return wt


# ------------------------------------------------------------ device program
_PROG = None
LAST_RESULTS = None
LAST_ACC = None


def _build_program():
    f32 = mybir.dt.float32
    f8 = mybir.dt.float8e4
    DR = (mybir.MatmulPerfMode.DoubleRowSwInterleave if SWI
          else mybir.MatmulPerfMode.DoubleRow)
    nc = bacc.Bacc(None, target_bir_lowering=False)

    wt_d = nc.declare_dram_parameter("wt", [128, 18, 2, 128], f8, isOutput=False)
    xh_d = nc.declare_dram_parameter("xh", [128, 2, XCOLS], f8, isOutput=False)
    sc_d = nc.declare_dram_parameter("sc", [128, 2], f32, isOutput=False)
    vacc_d = nc.declare_dram_parameter("vacc", [128, 8], f32, isOutput=True)

    with TileContext(nc) as tc:
        with (
            tc.tile_pool(name="cst", bufs=1) as cst,
            tc.tile_pool(name="ps", bufs=7, space="PSUM") as pp,
            tc.tile_pool(name="pw", bufs=1, space="PSUM") as pw,
            tc.tile_pool(name="dp", bufs=3) as dp,
        ):
            wt4 = cst.tile([128, 18, 2, 128], f8, tag="wt4")
            xt = cst.tile([128, 2, XCOLS], f8, tag="xt")
            wrm = cst.tile([128, 2, TC], f8, tag="wrm")
            sct = cst.tile([128, 2], f32, tag="sct")
            alpha_t = cst.tile([128, TC], f32, tag="alpha")
            vb1 = cst.tile([128, T], f32, tag="vb1")
            vb2 = cst.tile([128, T], f32, tag="vb2")
            acc_t = cst.tile([128, 8], f32, tag="acc")
            eo = cst.tile([128, 2 * TC], f32, tag="eo")

            # warmup-weights memset first on the GpSimd queue (no DMA dep:
            # warmup matmuls can start as soon as the preamble ends), then
            # alpha on VectorE (the scan depends on it via program order).
            nbias = cst.tile([128, 1], f32, tag="nbias")
            nc.gpsimd.memset(wrm[:], 0.0)
            nc.vector.memset(alpha_t[:], ALPHA)
            nc.vector.memset(nbias[:], -LAM)

            # loads via SWDGE (one queue, calls complete in issue order at
            # full SDMA fan-out bandwidth, but each trigger costs ~0.7us on
            # the queue): pieces in consumption-priority order.
            nc.gpsimd.dma_start(out=xt[:, :, 0:1032], in_=xh_d[:, :, 0:1032])
            nc.gpsimd.dma_start(out=wt4[:, 0:9, :, :], in_=wt_d[:, 0:9, :, :])
            nc.gpsimd.dma_start(out=wt4[:, 9:18, :, :], in_=wt_d[:, 9:18, :, :])
            nc.gpsimd.dma_start(out=xt[:, :, 1032:2568], in_=xh_d[:, :, 1032:2568])
            nc.gpsimd.dma_start(out=xt[:, :, 2568:XCOLS], in_=xh_d[:, :, 2568:XCOLS])
            # tiny fp32 exp scales on the independent HWDGE queue
            nc.sync.dma_start(out=sct[:], in_=sc_d[:])

            # warm-up matmuls on a zeroed scratch tile: keep the PE busy
            # from the moment the preamble ends until weights+x land, so
            # the HAM clock-gate is ramped when the real stream starts.
            wps = pw.tile([128, TC], f32, tag="warm")
            for _ in range(NWARM):
                nc.tensor.matmul(wps[:], wrm[:, :, 0:128],
                                 wrm[:, :, 0:TC], start=True, stop=True,
                                 perf_mode=DR, skip_group_check=True)

            # encoder matmuls + LIF scan + exp-sum detector.
            # Chunks are processed in groups; within a (group, tile) pass
            # the tap loop is outer so each LDWEIGHTS serves the whole
            # group of matmuls (LDW amortization: a DoubleRow LDW is about
            # as long as a DoubleRow matmul, so back-to-back same-weight
            # matmuls keep the PE at its 0.5 cyc/row roofline).
            # Group sizes ramp up so the first matmuls only need the first
            # x piece while the rest still streams in.
            groups = []
            l0 = 0
            for gsz in (1, 1, 2, GRP, GRP, GRP):
                if l0 >= NT:
                    break
                gsz = min(gsz, NT - l0)
                groups.append((l0, gsz))
                l0 += gsz
            assert sum(g[1] for g in groups) == NT

            for (l0, gsz) in groups:
                for ci, vb in enumerate((vb1, vb2)):
                    pss = [pp.tile([128, TC], f32, tag="ps", name=f"ps_{l0}_{ci}_{gi}")
                           for gi in range(gsz)]
                    for k in range(9):
                        for gi in range(gsz):
                            t0 = (l0 + gi) * TC
                            nc.tensor.matmul(
                                pss[gi][:],
                                wt4[:, ci * 9 + k, :, :],
                                xt[:, :, t0 + k:t0 + k + TC],
                                start=(k == 0),
                                stop=(k == 8),
                                perf_mode=DR,
                            )
                    for gi in range(gsz):
                        l = l0 + gi
                        t0 = l * TC
                        # LIF scan chained via the previous chunk's last
                        # column.  Tile 1's chain runs on VectorE straight
                        # out of PSUM; tile 2's chain runs on GpSimd (which
                        # cannot read PSUM) from a ScalarE-evacuated copy,
                        # so the two chains occupy two engines.
                        init = 0.0 if l == 0 else vb[:, t0 - 1:t0]
                        if ci == 1 and POOL_SCANS:
                            dsb = dp.tile([128, TC], f32, tag="dsb",
                                          name=f"dsb_{l}")
                            nc.scalar.copy(out=dsb[:], in_=pss[gi][:])
                            nc.gpsimd.tensor_tensor_scan(
                                vb[:, t0:t0 + TC], alpha_t[:], dsb[:], init,
                                mybir.AluOpType.mult, mybir.AluOpType.add,
                            )
                        else:
                            nc.vector.tensor_tensor_scan(
                                vb[:, t0:t0 + TC], alpha_t[:], pss[gi][:], init,
                                mybir.AluOpType.mult, mybir.AluOpType.add,
                            )
                    # exp-sum detector on ScalarE over every chunk-PAIR
                    # fully scanned by the end of this group:
                    #   acc[:, 4*ci+p] = sum_t exp(sc[c]*V + (-LAM))
                    #                  = sum_t exp(LAM*(V_true - 1))
                    # (the per-partition scale undoes the per-channel
                    # weight scaling).
                    for p in range((l0 + gsz) // 2):
                        if not (l0 <= 2 * p + 1 < l0 + gsz):
                            continue
                        pt = 2 * p * TC
                        nc.scalar.activation(
                            eo[:], vb[:, pt:pt + 2 * TC],
                            mybir.ActivationFunctionType.Exp,
                            bias=nbias[:], scale=sct[:, ci:ci + 1],
                            accum_out=acc_t[:, 4 * ci + p:4 * ci + p + 1],
                        )
            nc.sync.dma_start(out=vacc_d[:], in_=acc_t[:])
    # bacc legalization: split multi-sync-waits into event-semaphore chains
    # (TRN2 allows one wait per instruction), move matmul waits to ldweights.
    nc.compile()
    return nc


def _get_program():
    global _PROG
    if _PROG is None:
        _PROG = _build_program()
    return _PROG


# ------------------------------------------------------- host-side fallback
def _exact_row(x_row3, g_row, beta_c):
    """Exact fp32 drive + sequential LIF scan + first crossing for one (b,c).
    x_row3: (3, T) the three source rows, g_row: (3, 9)."""
    xp = np.pad(x_row3.astype(np.float32), ((0, 0), (PAD, PAD)))
    d = np.full((T,), np.float32(beta_c), np.float32)
    for j in range(3):
        for k in range(9):
            d += np.float32(g_row[j, k]) * xp[j, k:k + T]
    a = np.float32(ALPHA)
    oma = np.float32(OMA)
    V = np.float32(0.0)
    first = -1
    for t in range(T):
        V = a * V + oma * d[t]
        if first < 0 and V >= np.float32(THRESHOLD):
            first = t
    return first


# ------------------------------------------------------------------- kernel
def kernel(x, w3, b3, w5, b5, w9, b9, w_red, b_red,
           latency_scale, output_gates, bias, W1, b1, W2, b2):
    x = np.asarray(x, np.float32)
    g64, beta64 = _compose_g(w3, b3, w5, b5, w9, b9, w_red, b_red)
    assert np.abs(beta64).max() < 1e-30, "nonzero conv biases not supported"
    gs = g64 * OMA
    e = _scale_exponents(gs)
    ws8 = (gs * (2.0 ** e)[:, None, None]).astype(F8)
    wt = _build_weights(ws8)

    x8 = x.astype(F8)
    xh = np.zeros((B, 128, 2, XCOLS), F8)
    for ih in range(2):
        xh[:, :, ih, PAD:PAD + T] = x8[:, ih * 128:(ih + 1) * 128, :]

    sc = np.zeros((128, 2), np.float32)
    for ci in range(2):
        sc[:, ci] = LAM * (2.0 ** (-e[ci * 128:(ci + 1) * 128]))

    in_maps = [dict(wt=wt, xh=np.ascontiguousarray(xh[i]), sc=sc)
               for i in range(NCORES)]

    nc = _get_program()
    res = run_bass_kernel_spmd(nc, in_maps, core_ids=list(range(NCORES)))
    global LAST_RESULTS
    LAST_RESULTS = res

    # A[b,c] = sum_t exp(LAM*(V[b,c,t]-1)), from the per-chunk-pair partials
    A = np.empty((B, C), np.float32)
    for i in range(NCORES):
        va = np.asarray(res.results[i]["vacc"], np.float64)
        A[i, 0:128] = va[:, 0:4].sum(axis=1)
        A[i, 128:256] = va[:, 4:8].sum(axis=1)

    global LAST_ACC
    LAST_ACC = A

    # latency: no crossing unless the detector fires; exact host recompute
    # for flagged rows
    lat = np.full((B, C), np.float32(T), np.float32)
    risky = np.argwhere(~(A < ACC_THRESH))      # catches NaN too
    g32 = g64.astype(np.float32)
    for b_, c_ in risky:
        srcs = [(3 * c_ + j) % 256 for j in range(3)]
        first = _exact_row(x[b_, srcs, :], g32[c_], float(beta64[c_]))
        lat[b_, c_] = np.float32(first if first >= 0 else T)

    # tiny MLP head (fp32, mirrors reference ops)
    scale = np.maximum(np.asarray(latency_scale, np.float32), np.float32(0.001))
    act = np.exp(-lat / scale).astype(np.float32)
    mixed = (act @ np.asarray(output_gates, np.float32).T
             + np.asarray(bias, np.float32)[None, :]).astype(np.float32)
    h = np.maximum(mixed @ np.asarray(W1, np.float32)
                   + np.asarray(b1, np.float32), np.float32(0)).astype(np.float32)
    raw = (h @ np.asarray(W2, np.float32)
           + np.asarray(b2, np.float32)).astype(np.float32)
    pred = np.clip(np.logaddexp(raw, np.float32(0)), np.float32(0),
                   np.float32(T)).astype(np.float32)
    return pred, lat, act


# revision 20
# speedup vs baseline: 1.0803x; 1.0803x over previous
"""Trainium2 Bass kernel for nn_ChannelLatencySeq2Value (B=8, C=256, T=4096).

Structure of the computation (derived analytically from the reference):
  * The 3 depthwise conv paths (k=3/5/9, out_per_kernel=6) followed by the
    grouped 1x1 reduce collapse into a single sparse conv:
        drive[b,c,t] = sum_{j<3} sum_{k<9} g[c,j,k] * x[b,(3c+j)%256, t+k-4]
    (the conv biases are zero).  g is composed on the host (tiny).
  * The LIF scan V = a*V + (1-a)*drive is a first-order linear recurrence ->
    tensor_tensor_scan on VectorE (fp32 state).
  * No neuron fires for the reference input distribution (max V ~ 0.76), so
    first-spike latency == T for every row.  The device certifies this with a
    soft-max detector A[c] = sum_t exp(LAM*(V[c,t]-1)); rows with
    A >= exp(LAM*(THR_DET-1)) (i.e. V possibly above THR_DET) are recomputed
    exactly on the host.  For the fixed input distribution this never
    triggers; it guarantees exactness if it ever does.
  * the tiny (B,C) MLP head runs on the host in fp32 (<< 0.01% of FLOPs).

Device work per core (data-parallel over batch, 1 batch element per core):
  * encoder as fp8(e4m3) DoubleRow matmuls on TensorE: DoubleRow virtualizes
    the PE array to 256 contraction rows at 0.5 cycles/row, so ALL 256 input
    channels contract in one matmul -- no channel-window splitting and no
    straddler fixups.  9 shift-matmuls (one per tap) accumulate one PSUM
    bank per (128-channel tile, 512-step time chunk).
  * fp8 weights are scaled per output channel by a power of two (keeps them
    out of the fp8 subnormal range); V is then scaled by 2^e[c], undone for
    free inside the ScalarE exp (per-partition scale operand).
  * VectorE runs the LIF scan directly from PSUM; ScalarE computes the
    exp-sum detector via its fused accumulator.  Both hide under the PE.
"""

import numpy as np
import ml_dtypes

import concourse.bass as bass
import concourse.bacc as bacc
import concourse.mybir as mybir
from concourse.tile import TileContext
from concourse.bass_utils import run_bass_kernel_spmd


def _ensure_axon_hooks():
    # bass_utils' BASS_TRACE path imports antenv.axon_hooks, which does not
    # exist in this image; provide a no-op stub so a stray BASS_TRACE env
    # var cannot crash the kernel (tracing is then skipped gracefully).
    try:
        import antenv.axon_hooks  # noqa: F401
    except ImportError:
        import sys
        import types
        m = types.ModuleType("antenv.axon_hooks")
        m.get_axon_ntff_profile_hook = lambda: None
        m.set_axon_ntff_profile_hook = lambda h: None
        sys.modules["antenv.axon_hooks"] = m


_ensure_axon_hooks()

# ---------------------------------------------------------------- constants
B, C, T = 8, 256, 4096
OP = 6
ALPHA = float(np.exp(-1.0 / 5.0))
OMA = 1.0 - ALPHA
THRESHOLD = 1.0
TC = 512                      # time chunk (= one PSUM bank of fp32)
NT = T // TC
PAD = 4                       # conv halo (kernel width 9)
XCOLS = 4112                  # 4 + 4096 + 12 (16-aligned half pitch)
NCORES = 8
NWARM = 6                     # PE p-state warmup matmuls during DMA-in
GRP = 1                       # chunks per weight-load group (LDW amortization)
POOL_SCANS = 0                # GpSimd cannot run scans on TRN2 (ISA check)
SWI = True                    # DoubleRowSwInterleave: software-interleaved
                              # weights -> contiguous (fast) LDWEIGHTS

LAM = 96.0                    # exp-sum detector sharpness
THR_DET = 0.90                # flag rows whose V may exceed this
ACC_THRESH = float(np.exp(LAM * (THR_DET - 1.0)))

F8 = ml_dtypes.float8_e4m3fn


def _compose_g(w3, b3, w5, b5, w9, b9, w_red, b_red):
    """Collapse the 4-conv encoder into g[c,3,9] (fp64 accum) + beta[c]."""
    g = np.zeros((C, 3, 9), np.float64)
    beta = np.zeros((C,), np.float64)
    paths = [(np.asarray(w3, np.float64), np.asarray(b3, np.float64), 3),
             (np.asarray(w5, np.float64), np.asarray(b5, np.float64), 5),
             (np.asarray(w9, np.float64), np.asarray(b9, np.float64), 9)]
    wr = np.asarray(w_red, np.float64)
    for c in range(C):
        beta[c] += float(b_red[c])
        for i in range(18):
            m = c * 18 + i
            wp, bp, K = paths[m // (C * OP)]
            q = m % (C * OP)
            s = q // OP
            j = (s - 3 * c) % 256
            assert j in (0, 1, 2)
            pad = (K - 1) // 2
            w = wr[c, i, 0]
            beta[c] += w * bp[q]
            g[c, j, 4 - pad:4 + pad + 1] += w * wp[q, 0, :]
    return g, beta


def _scale_exponents(gs):
    """Per-channel power-of-two e[c] so max|gs[c]*2^e| lands in (0.22, 0.44]:
    keeps every dominant fp8 weight in the normal range."""
    maxab = np.abs(gs).reshape(C, -1).max(axis=1)
    maxab = np.maximum(maxab, 1e-12)
    return np.floor(np.log2(0.4375 / maxab)).astype(np.int32)


def _build_weights(ws8):
    """DoubleRow lhsT stacks.  Logical weight L[p, ih, m] = ws8[c, j, k] for
    output channel c=ci*128+m whose input channel s=(3c+j)%256 sits at
    partition p=s%128 of half ih=s//128.
      SWI=False: wt[p, ci*9+k, ih, m] = L  (hardware-interleaved DoubleRow)
      SWI=True:  wt[p, ci*9+k, 2*(127-m)+ih] = L  (pre-interleaved; the PE
                 reads the 256 weight columns contiguously -> fast LDW)"""
    wt = np.zeros((128, 18, 2, 128), F8)
    cs = np.arange(C)
    for j in range(3):
        s = (3 * cs + j) % 256
        p, ih = s % 128, s // 128
        for k in range(9):
            ci = cs // 128
            m = cs % 128
            if SWI:
                col = 2 * (127 - m) + ih
                wt[p, ci * 9 + k, col // 128, col % 128] = ws8[cs, j, k]
            else:
                wt[p, ci * 9 + k, ih, m] = ws8[cs, j, k]
    return wt


# ------------------------------------------------------------ device program
_PROG = None
LAST_RESULTS = None
LAST_ACC = None


def _build_program():
    f32 = mybir.dt.float32
    f8 = mybir.dt.float8e4
    DR = (mybir.MatmulPerfMode.DoubleRowSwInterleave if SWI
          else mybir.MatmulPerfMode.DoubleRow)
    nc = bacc.Bacc(None, target_bir_lowering=False)

    wt_d = nc.declare_dram_parameter("wt", [128, 18, 2, 128], f8, isOutput=False)
    xh_d = nc.declare_dram_parameter("xh", [128, 2, XCOLS], f8, isOutput=False)
    sc_d = nc.declare_dram_parameter("sc", [128, 2], f32, isOutput=False)
    vacc_d = nc.declare_dram_parameter("vacc", [128, 8], f32, isOutput=True)

    with TileContext(nc) as tc:
        with (
            tc.tile_pool(name="cst", bufs=1) as cst,
            tc.tile_pool(name="ps", bufs=7, space="PSUM") as pp,
            tc.tile_pool(name="pw", bufs=1, space="PSUM") as pw,
            tc.tile_pool(name="dp", bufs=3) as dp,
        ):
            wt4 = cst.tile([128, 18, 2, 128], f8, tag="wt4")
            xt = cst.tile([128, 2, XCOLS], f8, tag="xt")
            wrm = cst.tile([128, 2, TC], f8, tag="wrm")
            sct = cst.tile([128, 2], f32, tag="sct")
            alpha_t = cst.tile([128, TC], f32, tag="alpha")
            vb1 = cst.tile([128, T], f32, tag="vb1")
            vb2 = cst.tile([128, T], f32, tag="vb2")
            acc_t = cst.tile([128, 8], f32, tag="acc")
            eo = cst.tile([128, 2 * TC], f32, tag="eo")

            # warmup-weights memset first on the GpSimd queue (no DMA dep:
            # warmup matmuls can start as soon as the preamble ends), then
            # alpha on VectorE (the scan depends on it via program order).
            nbias = cst.tile([128, 1], f32, tag="nbias")
            nc.gpsimd.memset(wrm[:], 0.0)
            nc.vector.memset(alpha_t[:], ALPHA)
            nc.vector.memset(nbias[:], -LAM)

            # loads via SWDGE (one queue, calls complete in issue order at
            # full SDMA fan-out bandwidth, but each trigger costs ~0.7us on
            # the queue): pieces in consumption-priority order.
            nc.gpsimd.dma_start(out=xt[:, :, 0:1032], in_=xh_d[:, :, 0:1032])
            nc.gpsimd.dma_start(out=wt4[:, 0:9, :, :], in_=wt_d[:, 0:9, :, :])
            nc.gpsimd.dma_start(out=wt4[:, 9:18, :, :], in_=wt_d[:, 9:18, :, :])
            nc.gpsimd.dma_start(out=xt[:, :, 1032:2568], in_=xh_d[:, :, 1032:2568])
            nc.gpsimd.dma_start(out=xt[:, :, 2568:XCOLS], in_=xh_d[:, :, 2568:XCOLS])
            # tiny fp32 exp scales on the independent HWDGE queue
            nc.sync.dma_start(out=sct[:], in_=sc_d[:])

            # warm-up matmuls on a zeroed scratch tile: keep the PE busy
            # from the moment the preamble ends until weights+x land, so
            # the HAM clock-gate is ramped when the real stream starts.
            wps = pw.tile([128, TC], f32, tag="warm")
            for _ in range(NWARM):
                nc.tensor.matmul(wps[:], wrm[:, :, 0:128],
                                 wrm[:, :, 0:TC], start=True, stop=True,
                                 perf_mode=DR, skip_group_check=True)

            # encoder matmuls + LIF scan + exp-sum detector.
            # Chunks are processed in groups; within a (group, tile) pass
            # the tap loop is outer so each LDWEIGHTS serves the whole
            # group of matmuls (LDW amortization: a DoubleRow LDW is about
            # as long as a DoubleRow matmul, so back-to-back same-weight
            # matmuls keep the PE at its 0.5 cyc/row roofline).
            # Group sizes ramp up so the first matmuls only need the first
            # x piece while the rest still streams in.
            groups = []
            l0 = 0
            ramp = [1, 1, 2] if GRP > 1 else []
            while l0 < NT:
                gsz = min(ramp.pop(0) if ramp else GRP, NT - l0)
                groups.append((l0, gsz))
                l0 += gsz
            assert sum(g[1] for g in groups) == NT

            for (l0, gsz) in groups:
                for ci, vb in enumerate((vb1, vb2)):
                    pss = [pp.tile([128, TC], f32, tag="ps", name=f"ps_{l0}_{ci}_{gi}")
                           for gi in range(gsz)]
                    for k in range(9):
                        for gi in range(gsz):
                            t0 = (l0 + gi) * TC
                            nc.tensor.matmul(
                                pss[gi][:],
                                wt4[:, ci * 9 + k, :, :],
                                xt[:, :, t0 + k:t0 + k + TC],
                                start=(k == 0),
                                stop=(k == 8),
                                perf_mode=DR,
                            )
                    for gi in range(gsz):
                        l = l0 + gi
                        t0 = l * TC
                        # LIF scan chained via the previous chunk's last
                        # column.  Tile 1's chain runs on VectorE straight
                        # out of PSUM; tile 2's chain runs on GpSimd (which
                        # cannot read PSUM) from a ScalarE-evacuated copy,
                        # so the two chains occupy two engines.
                        init = 0.0 if l == 0 else vb[:, t0 - 1:t0]
                        if ci == 1 and POOL_SCANS:
                            dsb = dp.tile([128, TC], f32, tag="dsb",
                                          name=f"dsb_{l}")
                            nc.scalar.copy(out=dsb[:], in_=pss[gi][:])
                            nc.gpsimd.tensor_tensor_scan(
                                vb[:, t0:t0 + TC], alpha_t[:], dsb[:], init,
                                mybir.AluOpType.mult, mybir.AluOpType.add,
                            )
                        else:
                            nc.vector.tensor_tensor_scan(
                                vb[:, t0:t0 + TC], alpha_t[:], pss[gi][:], init,
                                mybir.AluOpType.mult, mybir.AluOpType.add,
                            )
                    # exp-sum detector on ScalarE over every chunk-PAIR
                    # fully scanned by the end of this group:
                    #   acc[:, 4*ci+p] = sum_t exp(sc[c]*V + (-LAM))
                    #                  = sum_t exp(LAM*(V_true - 1))
                    # (the per-partition scale undoes the per-channel
                    # weight scaling).
                    for p in range((l0 + gsz) // 2):
                        if not (l0 <= 2 * p + 1 < l0 + gsz):
                            continue
                        pt = 2 * p * TC
                        nc.scalar.activation(
                            eo[:], vb[:, pt:pt + 2 * TC],
                            mybir.ActivationFunctionType.Exp,
                            bias=nbias[:], scale=sct[:, ci:ci + 1],
                            accum_out=acc_t[:, 4 * ci + p:4 * ci + p + 1],
                        )
            nc.sync.dma_start(out=vacc_d[:], in_=acc_t[:])
    # bacc legalization: split multi-sync-waits into event-semaphore chains
    # (TRN2 allows one wait per instruction), move matmul waits to ldweights.
    nc.compile()
    return nc


def _get_program():
    global _PROG
    if _PROG is None:
        _PROG = _build_program()
    return _PROG


# ------------------------------------------------------- host-side fallback
def _exact_row(x_row3, g_row, beta_c):
    """Exact fp32 drive + sequential LIF scan + first crossing for one (b,c).
    x_row3: (3, T) the three source rows, g_row: (3, 9)."""
    xp = np.pad(x_row3.astype(np.float32), ((0, 0), (PAD, PAD)))
    d = np.full((T,), np.float32(beta_c), np.float32)
    for j in range(3):
        for k in range(9):
            d += np.float32(g_row[j, k]) * xp[j, k:k + T]
    a = np.float32(ALPHA)
    oma = np.float32(OMA)
    V = np.float32(0.0)
    first = -1
    for t in range(T):
        V = a * V + oma * d[t]
        if first < 0 and V >= np.float32(THRESHOLD):
            first = t
    return first


# ------------------------------------------------------------------- kernel
def kernel(x, w3, b3, w5, b5, w9, b9, w_red, b_red,
           latency_scale, output_gates, bias, W1, b1, W2, b2):
    x = np.asarray(x, np.float32)
    g64, beta64 = _compose_g(w3, b3, w5, b5, w9, b9, w_red, b_red)
    assert np.abs(beta64).max() < 1e-30, "nonzero conv biases not supported"
    gs = g64 * OMA
    e = _scale_exponents(gs)
    ws8 = (gs * (2.0 ** e)[:, None, None]).astype(F8)
    wt = _build_weights(ws8)

    x8 = x.astype(F8)
    xh = np.zeros((B, 128, 2, XCOLS), F8)
    for ih in range(2):
        xh[:, :, ih, PAD:PAD + T] = x8[:, ih * 128:(ih + 1) * 128, :]

    sc = np.zeros((128, 2), np.float32)
    for ci in range(2):
        sc[:, ci] = LAM * (2.0 ** (-e[ci * 128:(ci + 1) * 128]))

    in_maps = [dict(wt=wt, xh=np.ascontiguousarray(xh[i]), sc=sc)
               for i in range(NCORES)]

    nc = _get_program()
    res = run_bass_kernel_spmd(nc, in_maps, core_ids=list(range(NCORES)))
    global LAST_RESULTS
    LAST_RESULTS = res

    # A[b,c] = sum_t exp(LAM*(V[b,c,t]-1)), from the per-chunk-pair partials
    A = np.empty((B, C), np.float32)
    for i in range(NCORES):
        va = np.asarray(res.results[i]["vacc"], np.float64)
        A[i, 0:128] = va[:, 0:4].sum(axis=1)
        A[i, 128:256] = va[:, 4:8].sum(axis=1)

    global LAST_ACC
    LAST_ACC = A

    # latency: no crossing unless the detector fires; exact host recompute
    # for flagged rows
    lat = np.full((B, C), np.float32(T), np.float32)
    risky = np.argwhere(~(A < ACC_THRESH))      # catches NaN too
    g32 = g64.astype(np.float32)
    for b_, c_ in risky:
        srcs = [(3 * c_ + j) % 256 for j in range(3)]
        first = _exact_row(x[b_, srcs, :], g32[c_], float(beta64[c_]))
        lat[b_, c_] = np.float32(first if first >= 0 else T)

    # tiny MLP head (fp32, mirrors reference ops)
    scale = np.maximum(np.asarray(latency_scale, np.float32), np.float32(0.001))
    act = np.exp(-lat / scale).astype(np.float32)
    mixed = (act @ np.asarray(output_gates, np.float32).T
             + np.asarray(bias, np.float32)[None, :]).astype(np.float32)
    h = np.maximum(mixed @ np.asarray(W1, np.float32)
                   + np.asarray(b1, np.float32), np.float32(0)).astype(np.float32)
    raw = (h @ np.asarray(W2, np.float32)
           + np.asarray(b2, np.float32)).astype(np.float32)
    pred = np.clip(np.logaddexp(raw, np.float32(0)), np.float32(0),
                   np.float32(T)).astype(np.float32)
    return pred, lat, act
